# revision 2
# baseline (speedup 1.0000x reference)
"""EventEmbeddingModel Trainium2 kernel.

kernel(**inputs) takes the FULL (unsharded) inputs and returns the full
[B, D] float32 output.  Data-parallel over batch across the 8 NeuronCores;
the embedding table and LinearQ weights are replicated.

Two per-core programs (built once each, cached):

v2 (ragged, default): the host does *layout only* — batch rows are sorted
by history length and striped across cores; each row's valid history slots
(l < hist_len, plus the ent_id fallback row when hist_len == 0) are packed
back-to-back into fixed 128-row "gather groups".  Each output chunk of 128
rows has a compile-time layer budget Lc sized for the hist_len
distribution (uniform over [0, 64]), with ~2.5 layers of slack; if some
other distribution does not fit the skeleton the kernel falls back to v1.
The device does all arithmetic: w = exp(t - ct) per packed slot, a
one-instruction rhs build (column-match * weight) per group, two matmuls
per group accumulating his^T in PSUM, and the final linear
y = his @ W^T + b.  The host inverse-permutes the output rows.

v1 (dense fallback): every row processes all 64 slots; weights are masked
on device (iota < hist_len), the fallback is folded into slot 0, and
weight/index tiles are shuffled into pair layout with a PE transpose.

Both were validated on hardware against the jax reference (rel err ~3e-6).
"""
import sys

import numpy as np

if "/opt/trn_rl_repo" not in sys.path:
    sys.path.insert(0, "/opt/trn_rl_repo")

B, L, V, D = 8192, 64, 100000, 256
N_CORES = 8
BL = B // N_CORES
P = 128
NCHUNK = BL // P
NPAIR = L

# per-chunk gather-layer budgets for hist_len ~ U[0, 64] after sorting,
# ~= E[sum hist_len of chunk]/128 + 2.5 layers of slack
LC = (7, 15, 23, 31, 39, 47, 55, 63)
G_TOTAL = sum(LC)


def _common_io(nc, mybir):
    f32, i32 = mybir.dt.float32, mybir.dt.int32
    emb = nc.dram_tensor("emb", [V, D], f32, kind="ExternalInput").ap()
    wt_d = nc.dram_tensor("WT", [D, D], f32, kind="ExternalInput").ap()
    b_d = nc.dram_tensor("bvec", [D], f32, kind="ExternalInput").ap()
    y_d = nc.dram_tensor("y", [BL, D], f32, kind="ExternalOutput").ap()
    return emb, wt_d, b_d, y_d


def _final_consts(nc, tc, cpool, pt, wt_d, b_d, mybir):
    """W^T halves and a PE-broadcast bias tile."""
    f32 = mybir.dt.float32
    wt0 = cpool.tile([P, D], f32)
    wt1 = cpool.tile([P, D], f32)
    nc.sync.dma_start(out=wt0[:], in_=wt_d[0:P, :])
    nc.sync.dma_start(out=wt1[:], in_=wt_d[P:D, :])
    bias_row = cpool.tile([1, D], f32)
    nc.sync.dma_start(out=bias_row[:], in_=b_d[None, :])
    ones_row = cpool.tile([1, P], f32)
    nc.vector.memset(ones_row[:], 1.0)
    bias_ps = pt.tile([P, D], f32, tag="bias_ps")
    nc.tensor.matmul(out=bias_ps[:], lhsT=ones_row[:], rhs=bias_row[:],
                     start=True, stop=True)
    bias_t = cpool.tile([P, D], f32)
    nc.vector.tensor_copy(bias_t[:], bias_ps[:])
    return wt0, wt1, bias_t


def build_nc_v2(debug=False, reps=1):
    import concourse.bass as bass
    import concourse.tile as tile
    from concourse import bacc, mybir

    f32, i32 = mybir.dt.float32, mybir.dt.int32
    op = mybir.AluOpType
    act = mybir.ActivationFunctionType

    nc = bacc.Bacc("TRN2", target_bir_lowering=False, debug=debug,
                   num_devices=N_CORES)

    idx_d = nc.dram_tensor("idxg", [P, G_TOTAL], i32,
                           kind="ExternalInput").ap()
    ht_d = nc.dram_tensor("htg", [P, G_TOTAL], f32,
                          kind="ExternalInput").ap()
    ct_d = nc.dram_tensor("ctg", [P, G_TOTAL], f32,
                          kind="ExternalInput").ap()
    seg_d = nc.dram_tensor("segg", [P, G_TOTAL], f32,
                           kind="ExternalInput").ap()
    emb, wt_d, b_d, y_d = _common_io(nc, mybir)

    with tile.TileContext(nc) as tc:
        with tc.tile_pool(name="const", bufs=1) as cpool, \
             tc.tile_pool(name="stage", bufs=6) as stp, \
             tc.tile_pool(name="gather", bufs=12) as gp, \
             tc.tile_pool(name="outp", bufs=2) as outp, \
             tc.tile_pool(name="pt", bufs=1, space="PSUM") as pt, \
             tc.tile_pool(name="phis", bufs=2, space="PSUM") as phis, \
             tc.tile_pool(name="py", bufs=2, space="PSUM") as py:

            iota_i = cpool.tile([P, P], i32)
            nc.gpsimd.iota(iota_i[:], pattern=[[1, P]], base=0,
                           channel_multiplier=0)
            iota_f = cpool.tile([P, P], f32)
            nc.vector.tensor_copy(iota_f[:], iota_i[:])

            wt0, wt1, bias_t = _final_consts(nc, tc, cpool, pt, wt_d, b_d,
                                             mybir)

            idxg = cpool.tile([P, G_TOTAL], i32)
            nc.sync.dma_start(out=idxg[:], in_=idx_d[:, :])
            seg = cpool.tile([P, G_TOTAL], f32)
            nc.sync.dma_start(out=seg[:], in_=seg_d[:, :])
            htg = cpool.tile([P, G_TOTAL], f32)
            nc.sync.dma_start(out=htg[:], in_=ht_d[:, :])
            ctg = cpool.tile([P, G_TOTAL], f32)
            nc.sync.dma_start(out=ctg[:], in_=ct_d[:, :])
            wg = cpool.tile([P, G_TOTAL], f32)
            nc.vector.tensor_tensor(out=wg[:], in0=htg[:], in1=ctg[:],
                                    op=op.subtract)
            nc.scalar.activation(out=wg[:], in_=wg[:], func=act.Exp,
                                 bias=0.0, scale=1.0)

            for _rep in range(reps):
                gbase = 0
                for c in range(NCHUNK):
                    r0, r1 = c * P, (c + 1) * P
                    lc = LC[c]

                    hisT0 = phis.tile([P, P], f32)
                    hisT1 = phis.tile([P, P], f32)
                    for j in range(lc):
                        col = gbase + j
                        g = gp.tile([P, D], f32, tag="g")
                        nc.gpsimd.indirect_dma_start(
                            out=g[:], out_offset=None, in_=emb[:],
                            in_offset=bass.IndirectOffsetOnAxis(
                                ap=idxg[:, col:col + 1], axis=0))
                        rhs_g = stp.tile([P, P], f32, tag="rhs")
                        nc.vector.tensor_scalar(
                            rhs_g[:], iota_f[:], seg[:, col:col + 1],
                            wg[:, col:col + 1], op.is_equal, op.mult)
                        nc.tensor.matmul(
                            out=hisT0[:], lhsT=g[:, 0:P], rhs=rhs_g[:],
                            start=(j == 0), stop=(j == lc - 1))
                        nc.tensor.matmul(
                            out=hisT1[:], lhsT=g[:, P:D], rhs=rhs_g[:],
                            start=(j == 0), stop=(j == lc - 1))
                    gbase += lc

                    hisT0_sb = outp.tile([P, P], f32)
                    nc.vector.tensor_copy(hisT0_sb[:], hisT0[:])
                    hisT1_sb = outp.tile([P, P], f32)
                    nc.vector.tensor_copy(hisT1_sb[:], hisT1[:])

                    y_ps = py.tile([P, D], f32)
                    nc.tensor.matmul(out=y_ps[:], lhsT=hisT0_sb[:],
                                     rhs=wt0[:], start=True, stop=False)
                    nc.tensor.matmul(out=y_ps[:], lhsT=hisT1_sb[:],
                                     rhs=wt1[:], start=False, stop=True)

                    y_sb = outp.tile([P, D], f32)
                    nc.vector.tensor_tensor(out=y_sb[:], in0=y_ps[:],
                                            in1=bias_t[:], op=op.add)
                    nc.sync.dma_start(out=y_d[r0:r1, :], in_=y_sb[:])

    nc.compile()
    return nc


def pack_v2(ent_ids, current_time, hist_ids, hist_times, hist_len):
    """Host-side layout: sort by length, stripe across cores, pack valid
    slots into the fixed [128, G_TOTAL] gather skeleton.

    Returns (per_core_packs, perm) or (None, None) if the skeleton does
    not fit this data (caller falls back to the dense kernel)."""
    hl = np.asarray(hist_len, dtype=np.int64)
    hl_adj = np.maximum(hl, 1)
    order = np.argsort(hl_adj, kind="stable")

    packs = []
    for c in range(N_CORES):
        rows = order[c::N_CORES]  # 1024 batch indices, ascending hist_len
        idxg = np.zeros((P, G_TOTAL), np.int32)
        htg = np.zeros((P, G_TOTAL), np.float32)
        ctg = np.zeros((P, G_TOTAL), np.float32)
        segg = np.full((P, G_TOTAL), -1.0, np.float32)
        gbase = 0
        for ch in range(NCHUNK):
            lc = LC[ch]
            cap = P * lc
            bidx = rows[ch * P:(ch + 1) * P]
            counts = hl_adj[bidx]
            total = int(counts.sum())
            if total > cap:
                return None, None
            seg_s = np.repeat(np.arange(P, dtype=np.float32), counts)
            ct_s = np.repeat(current_time[bidx].astype(np.float32), counts)
            idx_parts, ht_parts = [], []
            for k, b in enumerate(bidx):
                n = hl[b]
                if n > 0:
                    idx_parts.append(hist_ids[b, :n])
                    ht_parts.append(hist_times[b, :n])
                else:
                    idx_parts.append(np.array([ent_ids[b]], np.int32))
                    ht_parts.append(
                        np.array([current_time[b]], np.float32))
            idx_s = np.concatenate(idx_parts).astype(np.int32)
            ht_s = np.concatenate(ht_parts).astype(np.float32)

            pad = cap - total
            if pad:
                idx_s = np.pad(idx_s, (0, pad))
                ht_s = np.pad(ht_s, (0, pad))
                ct_s = np.pad(ct_s, (0, pad))
                seg_s = np.pad(seg_s, (0, pad), constant_values=-1.0)
            sl = slice(gbase, gbase + lc)
            idxg[:, sl] = idx_s.reshape(lc, P).T
            htg[:, sl] = ht_s.reshape(lc, P).T
            ctg[:, sl] = ct_s.reshape(lc, P).T
            segg[:, sl] = seg_s.reshape(lc, P).T
            gbase += lc
        packs.append({"idxg": idxg, "htg": htg, "ctg": ctg, "segg": segg})
    return packs, order


# ---------------------------------------------------------------------------
# v1 dense fallback
# ---------------------------------------------------------------------------

def build_nc_v1(debug=False, reps=1):
    import concourse.bass as bass
    import concourse.tile as tile
    from concourse import bacc, mybir
    from concourse.masks import make_identity

    f32, i32 = mybir.dt.float32, mybir.dt.int32
    op = mybir.AluOpType
    act = mybir.ActivationFunctionType

    nc = bacc.Bacc("TRN2", target_bir_lowering=False, debug=debug,
                   num_devices=N_CORES)

    ent = nc.dram_tensor("ent_ids", [BL], i32, kind="ExternalInput").ap()
    ct_d = nc.dram_tensor("current_time", [BL], f32,
                          kind="ExternalInput").ap()
    hid = nc.dram_tensor("hist_ids", [BL, L], i32, kind="ExternalInput").ap()
    ht_d = nc.dram_tensor("hist_times", [BL, L], f32,
                          kind="ExternalInput").ap()
    hl_d = nc.dram_tensor("hist_len", [BL], i32, kind="ExternalInput").ap()
    emb, wt_d, b_d, y_d = _common_io(nc, mybir)

    with tile.TileContext(nc) as tc:
        with tc.tile_pool(name="const", bufs=1) as cpool, \
             tc.tile_pool(name="io", bufs=2) as iop, \
             tc.tile_pool(name="stage", bufs=2) as stp, \
             tc.tile_pool(name="gather", bufs=8) as gp, \
             tc.tile_pool(name="outp", bufs=2) as outp, \
             tc.tile_pool(name="pt", bufs=1, space="PSUM") as pt, \
             tc.tile_pool(name="phis", bufs=2, space="PSUM") as phis, \
             tc.tile_pool(name="py", bufs=2, space="PSUM") as py:

            ident = cpool.tile([P, P], f32)
            make_identity(nc, ident[:])

            iota64_i = cpool.tile([P, L], i32)
            nc.gpsimd.iota(iota64_i[:], pattern=[[1, L]], base=0,
                           channel_multiplier=0)
            iota64_f = cpool.tile([P, L], f32)
            nc.vector.tensor_copy(iota64_f[:], iota64_i[:])

            iotap_i = cpool.tile([P, 1], i32)
            nc.gpsimd.iota(iotap_i[:], pattern=[[0, 1]], base=0,
                           channel_multiplier=1)
            iotap_f = cpool.tile([P, 1], f32)
            nc.vector.tensor_copy(iotap_f[:], iotap_i[:])

            halfmask = cpool.tile([P, 2], f32)
            nc.vector.tensor_scalar(halfmask[:, 0:1], iotap_f[:], 64.0, None,
                                    op.is_lt)
            nc.vector.tensor_scalar(halfmask[:, 1:2], iotap_f[:], 63.0, None,
                                    op.is_gt)

            wt0, wt1, bias_t = _final_consts(nc, tc, cpool, pt, wt_d, b_d,
                                             mybir)

            for _rep in range(reps):
                for c in range(NCHUNK):
                    r0, r1 = c * P, (c + 1) * P

                    idx_nat = iop.tile([P, L], i32)
                    nc.sync.dma_start(out=idx_nat[:], in_=hid[r0:r1, :])
                    ht = iop.tile([P, L], f32)
                    nc.sync.dma_start(out=ht[:], in_=ht_d[r0:r1, :])
                    ct = iop.tile([P, 1], f32)
                    nc.sync.dma_start(out=ct[:], in_=ct_d[r0:r1, None])
                    hl_i = iop.tile([P, 1], i32)
                    nc.sync.dma_start(out=hl_i[:], in_=hl_d[r0:r1, None])
                    eid = iop.tile([P, 1], i32)
                    nc.sync.dma_start(out=eid[:], in_=ent[r0:r1, None])

                    nct = stp.tile([P, 1], f32)
                    nc.vector.tensor_scalar_mul(nct[:], ct[:], -1.0)
                    hl_f = stp.tile([P, 1], f32)
                    nc.vector.tensor_copy(hl_f[:], hl_i[:])

                    wdup = stp.tile([P, 2 * L], f32)
                    nc.scalar.activation(out=wdup[:, 0:L], in_=ht[:],
                                         func=act.Exp, bias=nct[:], scale=1.0)
                    mask = stp.tile([P, L], f32)
                    nc.vector.tensor_scalar(mask[:], iota64_f[:], hl_f[:],
                                            None, op.is_lt)
                    nc.vector.tensor_tensor(out=wdup[:, 0:L],
                                            in0=wdup[:, 0:L], in1=mask[:],
                                            op=op.mult)
                    m_f = stp.tile([P, 1], f32)
                    nc.vector.tensor_scalar(m_f[:], hl_f[:], 0.0, None,
                                            op.is_equal)
                    nc.vector.tensor_tensor(out=wdup[:, 0:1],
                                            in0=wdup[:, 0:1], in1=m_f[:],
                                            op=op.add)
                    nc.vector.tensor_copy(wdup[:, L:2 * L], wdup[:, 0:L])

                    m_i = stp.tile([P, 1], i32)
                    nc.vector.tensor_scalar(m_i[:], hl_i[:], 0, None,
                                            op.is_equal)
                    nc.vector.copy_predicated(out=idx_nat[:, 0:1],
                                              mask=m_i[:], data=eid[:])

                    idxdup = stp.tile([P, 2 * L], f32)
                    nc.vector.tensor_copy(idxdup[:, 0:L], idx_nat[:])
                    nc.vector.tensor_copy(idxdup[:, L:2 * L], idx_nat[:])

                    t_w = pt.tile([P, P], f32, tag="tw")
                    nc.tensor.transpose(out=t_w[:], in_=wdup[:],
                                        identity=ident[:])
                    t_i = pt.tile([P, P], f32, tag="ti")
                    nc.tensor.transpose(out=t_i[:], in_=idxdup[:],
                                        identity=ident[:])

                    w_shuf = stp.tile([P, L], f32)
                    nc.vector.tensor_copy(w_shuf[0:64, :], t_w[0:64, 0:P:2])
                    nc.vector.tensor_copy(w_shuf[64:P, :], t_w[64:P, 1:P:2])
                    idx_shuf_f = stp.tile([P, L], f32)
                    nc.vector.tensor_copy(idx_shuf_f[0:64, :],
                                          t_i[0:64, 0:P:2])
                    nc.vector.tensor_copy(idx_shuf_f[64:P, :],
                                          t_i[64:P, 1:P:2])
                    idx_shuf = stp.tile([P, L], i32)
                    nc.vector.tensor_copy(idx_shuf[:], idx_shuf_f[:])

                    rhs_full = stp.tile([P, 2 * L], f32)
                    nc.vector.tensor_tensor(
                        out=rhs_full[:].rearrange("p (j n) -> p j n", n=2),
                        in0=w_shuf[:, :, None].to_broadcast([P, L, 2]),
                        in1=halfmask[:, None, :].to_broadcast([P, L, 2]),
                        op=op.mult)

                    hisT0 = phis.tile([P, P], f32)
                    hisT1 = phis.tile([P, P], f32)

                    for J in range(NPAIR):
                        g = gp.tile([P, D], f32, tag="g")
                        nc.gpsimd.indirect_dma_start(
                            out=g[:], out_offset=None, in_=emb[:],
                            in_offset=bass.IndirectOffsetOnAxis(
                                ap=idx_shuf[:, J:J + 1], axis=0))
                        nc.tensor.matmul(
                            out=hisT0[:, 2 * J:2 * J + 2], lhsT=g[:, 0:P],
                            rhs=rhs_full[:, 2 * J:2 * J + 2],
                            start=True, stop=True)
                        nc.tensor.matmul(
                            out=hisT1[:, 2 * J:2 * J + 2], lhsT=g[:, P:D],
                            rhs=rhs_full[:, 2 * J:2 * J + 2],
                            start=True, stop=True)

                    hisT0_sb = outp.tile([P, P], f32)
                    nc.vector.tensor_copy(hisT0_sb[:], hisT0[:])
                    hisT1_sb = outp.tile([P, P], f32)
                    nc.vector.tensor_copy(hisT1_sb[:], hisT1[:])

                    y_ps = py.tile([P, D], f32)
                    nc.tensor.matmul(out=y_ps[:], lhsT=hisT0_sb[:],
                                     rhs=wt0[:], start=True, stop=False)
                    nc.tensor.matmul(out=y_ps[:], lhsT=hisT1_sb[:],
                                     rhs=wt1[:], start=False, stop=True)

                    y_sb = outp.tile([P, D], f32)
                    nc.vector.tensor_tensor(out=y_sb[:], in0=y_ps[:],
                                            in1=bias_t[:], op=op.add)
                    nc.sync.dma_start(out=y_d[r0:r1, :], in_=y_sb[:])

    nc.compile()
    return nc


_NC_CACHE = {}


def _get_nc(which):
    if which not in _NC_CACHE:
        _NC_CACHE[which] = (build_nc_v2() if which == "v2"
                            else build_nc_v1())
    return _NC_CACHE[which]


def _norm_inputs(ent_ids, current_time, hist_ids, hist_times, hist_len,
                 emb, W, b):
    return (
        np.ascontiguousarray(np.asarray(ent_ids, dtype=np.int32)),
        np.ascontiguousarray(np.asarray(current_time, np.float32)),
        np.ascontiguousarray(np.asarray(hist_ids, dtype=np.int32)),
        np.ascontiguousarray(np.asarray(hist_times, np.float32)),
        np.ascontiguousarray(np.asarray(hist_len, dtype=np.int32)),
        np.ascontiguousarray(np.asarray(emb, dtype=np.float32)),
        np.ascontiguousarray(np.asarray(W, dtype=np.float32)),
        np.ascontiguousarray(np.asarray(b, dtype=np.float32)),
    )


def make_in_maps(ent_ids, current_time, hist_ids, hist_times, hist_len,
                 emb, W, b):
    """v1 (dense) per-core input maps."""
    ent_ids, current_time, hist_ids, hist_times, hist_len, emb, W, b = \
        _norm_inputs(ent_ids, current_time, hist_ids, hist_times, hist_len,
                     emb, W, b)
    WT = np.ascontiguousarray(W.T)
    in_maps = []
    for c in range(N_CORES):
        s = slice(c * BL, (c + 1) * BL)
        in_maps.append({
            "ent_ids": ent_ids[s], "current_time": current_time[s],
            "hist_ids": hist_ids[s], "hist_times": hist_times[s],
            "hist_len": hist_len[s], "emb": emb, "WT": WT, "bvec": b,
        })
    return in_maps


def make_in_maps_v2(ent_ids, current_time, hist_ids, hist_times, hist_len,
                    emb, W, b):
    """v2 (ragged) per-core input maps + output permutation, or (None, None)."""
    ent_ids, current_time, hist_ids, hist_times, hist_len, emb, W, b = \
        _norm_inputs(ent_ids, current_time, hist_ids, hist_times, hist_len,
                     emb, W, b)
    packs, order = pack_v2(ent_ids, current_time, hist_ids, hist_times,
                           hist_len)
    if packs is None:
        return None, None
    WT = np.ascontiguousarray(W.T)
    in_maps = []
    for c in range(N_CORES):
        m = dict(packs[c])
        m.update({"emb": emb, "WT": WT, "bvec": b})
        in_maps.append(m)
    return in_maps, order


def kernel(ent_ids, current_time, hist_ids, hist_times, hist_len, emb, W, b):
    from concourse.bass_utils import run_bass_kernel_spmd

    args = (ent_ids, current_time, hist_ids, hist_times, hist_len, emb, W, b)
    in_maps, order = make_in_maps_v2(*args)
    if in_maps is not None:
        nc = _get_nc("v2")
        res = run_bass_kernel_spmd(nc, in_maps, list(range(N_CORES)))
        y_sorted = np.stack([res.results[c]["y"] for c in range(N_CORES)])
        # core c position p holds batch row order[8p + c]
        y_full = np.empty((B, D), np.float32)
        pos = np.arange(BL)
        for c in range(N_CORES):
            y_full[order[N_CORES * pos + c]] = y_sorted[c]
        return y_full

    nc = _get_nc("v1")
    res = run_bass_kernel_spmd(nc, make_in_maps(*args),
                               list(range(N_CORES)))
    return np.concatenate([res.results[c]["y"] for c in range(N_CORES)],
                          axis=0)


# revision 3
# speedup vs baseline: 1.0482x; 1.0482x over previous
"""EventEmbeddingModel Trainium2 kernel.

kernel(**inputs) takes the FULL (unsharded) inputs and returns the full
[B, D] float32 output.  Data-parallel over batch across the 8 NeuronCores;
the embedding table and LinearQ weights are replicated.

Two per-core programs (built once each, cached):

v2 (ragged, default): the host does *layout only* — batch rows are sorted
by history length and striped across cores; each row's valid history slots
(l < hist_len, plus the ent_id fallback row when hist_len == 0) are packed
back-to-back into fixed 128-row "gather groups".  Each output chunk of 128
rows has a compile-time layer budget Lc sized for the hist_len
distribution (uniform over [0, 64]), with ~2.5 layers of slack; if some
other distribution does not fit the skeleton the kernel falls back to v1.
The device does all arithmetic: w = exp(t - ct) per packed slot, a
one-instruction rhs build (column-match * weight) per group, two matmuls
per group accumulating his^T in PSUM, and the final linear
y = his @ W^T + b.  The host inverse-permutes the output rows.

v1 (dense fallback): every row processes all 64 slots; weights are masked
on device (iota < hist_len), the fallback is folded into slot 0, and
weight/index tiles are shuffled into pair layout with a PE transpose.

Both were validated on hardware against the jax reference (rel err ~3e-6).
"""
import sys

import numpy as np

if "/opt/trn_rl_repo" not in sys.path:
    sys.path.insert(0, "/opt/trn_rl_repo")

B, L, V, D = 8192, 64, 100000, 256
N_CORES = 8
BL = B // N_CORES
P = 128
NCHUNK = BL // P
NPAIR = L

# per-chunk gather-layer budgets for hist_len ~ U[0, 64] after sorting,
# ~= E[sum hist_len of chunk]/128 + 2.5 layers of slack
LC = (7, 15, 23, 31, 39, 47, 55, 63)
G_TOTAL = sum(LC)


def _common_io(nc, mybir):
    f32, i32 = mybir.dt.float32, mybir.dt.int32
    emb = nc.dram_tensor("emb", [V, D], f32, kind="ExternalInput").ap()
    wt_d = nc.dram_tensor("WT", [D, D], f32, kind="ExternalInput").ap()
    b_d = nc.dram_tensor("bvec", [D], f32, kind="ExternalInput").ap()
    y_d = nc.dram_tensor("y", [BL, D], f32, kind="ExternalOutput").ap()
    return emb, wt_d, b_d, y_d


def _final_consts(nc, tc, cpool, pt, wt_d, b_d, mybir):
    """W^T halves and a PE-broadcast bias tile."""
    f32 = mybir.dt.float32
    wt0 = cpool.tile([P, D], f32)
    wt1 = cpool.tile([P, D], f32)
    nc.sync.dma_start(out=wt0[:], in_=wt_d[0:P, :])
    nc.sync.dma_start(out=wt1[:], in_=wt_d[P:D, :])
    bias_row = cpool.tile([1, D], f32)
    nc.sync.dma_start(out=bias_row[:], in_=b_d[None, :])
    ones_row = cpool.tile([1, P], f32)
    nc.vector.memset(ones_row[:], 1.0)
    bias_ps = pt.tile([P, D], f32, tag="bias_ps")
    nc.tensor.matmul(out=bias_ps[:], lhsT=ones_row[:], rhs=bias_row[:],
                     start=True, stop=True)
    bias_t = cpool.tile([P, D], f32)
    nc.vector.tensor_copy(bias_t[:], bias_ps[:])
    return wt0, wt1, bias_t


def build_nc_v2(debug=False, reps=1):
    import concourse.bass as bass
    import concourse.tile as tile
    from concourse import bacc, mybir

    f32, i32 = mybir.dt.float32, mybir.dt.int32
    op = mybir.AluOpType
    act = mybir.ActivationFunctionType

    # 280 indirect gathers x 256 ring descriptors each; the default 16KB
    # SWDGE carveout holds only ~4 gathers of descriptors, stalling Q7
    # descriptor generation on ring reclaim. 128KB keeps ~32 in flight.
    nc = bacc.Bacc("TRN2", target_bir_lowering=False, debug=debug,
                   num_devices=N_CORES, dynamic_dma_scratch_size=131072)

    idx_d = nc.dram_tensor("idxg", [P, G_TOTAL], i32,
                           kind="ExternalInput").ap()
    ht_d = nc.dram_tensor("htg", [P, G_TOTAL], f32,
                          kind="ExternalInput").ap()
    ct_d = nc.dram_tensor("ctg", [P, G_TOTAL], f32,
                          kind="ExternalInput").ap()
    seg_d = nc.dram_tensor("segg", [P, G_TOTAL], f32,
                           kind="ExternalInput").ap()
    emb, wt_d, b_d, y_d = _common_io(nc, mybir)

    with tile.TileContext(nc) as tc:
        with tc.tile_pool(name="const", bufs=1) as cpool, \
             tc.tile_pool(name="stage", bufs=6) as stp, \
             tc.tile_pool(name="gather", bufs=12) as gp, \
             tc.tile_pool(name="outp", bufs=2) as outp, \
             tc.tile_pool(name="pt", bufs=1, space="PSUM") as pt, \
             tc.tile_pool(name="phis", bufs=2, space="PSUM") as phis, \
             tc.tile_pool(name="py", bufs=2, space="PSUM") as py:

            iota_i = cpool.tile([P, P], i32)
            nc.gpsimd.iota(iota_i[:], pattern=[[1, P]], base=0,
                           channel_multiplier=0)
            iota_f = cpool.tile([P, P], f32)
            nc.vector.tensor_copy(iota_f[:], iota_i[:])

            wt0, wt1, bias_t = _final_consts(nc, tc, cpool, pt, wt_d, b_d,
                                             mybir)

            idxg = cpool.tile([P, G_TOTAL], i32)
            nc.sync.dma_start(out=idxg[:], in_=idx_d[:, :])
            seg = cpool.tile([P, G_TOTAL], f32)
            nc.sync.dma_start(out=seg[:], in_=seg_d[:, :])
            htg = cpool.tile([P, G_TOTAL], f32)
            nc.sync.dma_start(out=htg[:], in_=ht_d[:, :])
            ctg = cpool.tile([P, G_TOTAL], f32)
            nc.sync.dma_start(out=ctg[:], in_=ct_d[:, :])
            wg = cpool.tile([P, G_TOTAL], f32)
            nc.vector.tensor_tensor(out=wg[:], in0=htg[:], in1=ctg[:],
                                    op=op.subtract)
            nc.scalar.activation(out=wg[:], in_=wg[:], func=act.Exp,
                                 bias=0.0, scale=1.0)

            for _rep in range(reps):
                gbase = 0
                for c in range(NCHUNK):
                    r0, r1 = c * P, (c + 1) * P
                    lc = LC[c]

                    hisT0 = phis.tile([P, P], f32)
                    hisT1 = phis.tile([P, P], f32)
                    for j in range(lc):
                        col = gbase + j
                        g = gp.tile([P, D], f32, tag="g")
                        nc.gpsimd.indirect_dma_start(
                            out=g[:], out_offset=None, in_=emb[:],
                            in_offset=bass.IndirectOffsetOnAxis(
                                ap=idxg[:, col:col + 1], axis=0))
                        rhs_g = stp.tile([P, P], f32, tag="rhs")
                        nc.vector.tensor_scalar(
                            rhs_g[:], iota_f[:], seg[:, col:col + 1],
                            wg[:, col:col + 1], op.is_equal, op.mult)
                        nc.tensor.matmul(
                            out=hisT0[:], lhsT=g[:, 0:P], rhs=rhs_g[:],
                            start=(j == 0), stop=(j == lc - 1))
                        nc.tensor.matmul(
                            out=hisT1[:], lhsT=g[:, P:D], rhs=rhs_g[:],
                            start=(j == 0), stop=(j == lc - 1))
                    gbase += lc

                    hisT0_sb = outp.tile([P, P], f32)
                    nc.vector.tensor_copy(hisT0_sb[:], hisT0[:])
                    hisT1_sb = outp.tile([P, P], f32)
                    nc.vector.tensor_copy(hisT1_sb[:], hisT1[:])

                    y_ps = py.tile([P, D], f32)
                    nc.tensor.matmul(out=y_ps[:], lhsT=hisT0_sb[:],
                                     rhs=wt0[:], start=True, stop=False)
                    nc.tensor.matmul(out=y_ps[:], lhsT=hisT1_sb[:],
                                     rhs=wt1[:], start=False, stop=True)

                    y_sb = outp.tile([P, D], f32)
                    nc.vector.tensor_tensor(out=y_sb[:], in0=y_ps[:],
                                            in1=bias_t[:], op=op.add)
                    nc.sync.dma_start(out=y_d[r0:r1, :], in_=y_sb[:])

    nc.compile()
    return nc


def pack_v2(ent_ids, current_time, hist_ids, hist_times, hist_len):
    """Host-side layout: sort by length, stripe across cores, pack valid
    slots into the fixed [128, G_TOTAL] gather skeleton.

    Returns (per_core_packs, perm) or (None, None) if the skeleton does
    not fit this data (caller falls back to the dense kernel)."""
    hl = np.asarray(hist_len, dtype=np.int64)
    hl_adj = np.maximum(hl, 1)
    order = np.argsort(hl_adj, kind="stable")

    packs = []
    for c in range(N_CORES):
        rows = order[c::N_CORES]  # 1024 batch indices, ascending hist_len
        idxg = np.zeros((P, G_TOTAL), np.int32)
        htg = np.zeros((P, G_TOTAL), np.float32)
        ctg = np.zeros((P, G_TOTAL), np.float32)
        segg = np.full((P, G_TOTAL), -1.0, np.float32)
        gbase = 0
        for ch in range(NCHUNK):
            lc = LC[ch]
            cap = P * lc
            bidx = rows[ch * P:(ch + 1) * P]
            counts = hl_adj[bidx]
            total = int(counts.sum())
            if total > cap:
                return None, None
            seg_s = np.repeat(np.arange(P, dtype=np.float32), counts)
            ct_s = np.repeat(current_time[bidx].astype(np.float32), counts)
            idx_parts, ht_parts = [], []
            for k, b in enumerate(bidx):
                n = hl[b]
                if n > 0:
                    idx_parts.append(hist_ids[b, :n])
                    ht_parts.append(hist_times[b, :n])
                else:
                    idx_parts.append(np.array([ent_ids[b]], np.int32))
                    ht_parts.append(
                        np.array([current_time[b]], np.float32))
            idx_s = np.concatenate(idx_parts).astype(np.int32)
            ht_s = np.concatenate(ht_parts).astype(np.float32)

            pad = cap - total
            if pad:
                idx_s = np.pad(idx_s, (0, pad))
                ht_s = np.pad(ht_s, (0, pad))
                ct_s = np.pad(ct_s, (0, pad))
                seg_s = np.pad(seg_s, (0, pad), constant_values=-1.0)
            sl = slice(gbase, gbase + lc)
            idxg[:, sl] = idx_s.reshape(lc, P).T
            htg[:, sl] = ht_s.reshape(lc, P).T
            ctg[:, sl] = ct_s.reshape(lc, P).T
            segg[:, sl] = seg_s.reshape(lc, P).T
            gbase += lc
        packs.append({"idxg": idxg, "htg": htg, "ctg": ctg, "segg": segg})
    return packs, order


# ---------------------------------------------------------------------------
# v1 dense fallback
# ---------------------------------------------------------------------------

def build_nc_v1(debug=False, reps=1):
    import concourse.bass as bass
    import concourse.tile as tile
    from concourse import bacc, mybir
    from concourse.masks import make_identity

    f32, i32 = mybir.dt.float32, mybir.dt.int32
    op = mybir.AluOpType
    act = mybir.ActivationFunctionType

    nc = bacc.Bacc("TRN2", target_bir_lowering=False, debug=debug,
                   num_devices=N_CORES)

    ent = nc.dram_tensor("ent_ids", [BL], i32, kind="ExternalInput").ap()
    ct_d = nc.dram_tensor("current_time", [BL], f32,
                          kind="ExternalInput").ap()
    hid = nc.dram_tensor("hist_ids", [BL, L], i32, kind="ExternalInput").ap()
    ht_d = nc.dram_tensor("hist_times", [BL, L], f32,
                          kind="ExternalInput").ap()
    hl_d = nc.dram_tensor("hist_len", [BL], i32, kind="ExternalInput").ap()
    emb, wt_d, b_d, y_d = _common_io(nc, mybir)

    with tile.TileContext(nc) as tc:
        with tc.tile_pool(name="const", bufs=1) as cpool, \
             tc.tile_pool(name="io", bufs=2) as iop, \
             tc.tile_pool(name="stage", bufs=2) as stp, \
             tc.tile_pool(name="gather", bufs=8) as gp, \
             tc.tile_pool(name="outp", bufs=2) as outp, \
             tc.tile_pool(name="pt", bufs=1, space="PSUM") as pt, \
             tc.tile_pool(name="phis", bufs=2, space="PSUM") as phis, \
             tc.tile_pool(name="py", bufs=2, space="PSUM") as py:

            ident = cpool.tile([P, P], f32)
            make_identity(nc, ident[:])

            iota64_i = cpool.tile([P, L], i32)
            nc.gpsimd.iota(iota64_i[:], pattern=[[1, L]], base=0,
                           channel_multiplier=0)
            iota64_f = cpool.tile([P, L], f32)
            nc.vector.tensor_copy(iota64_f[:], iota64_i[:])

            iotap_i = cpool.tile([P, 1], i32)
            nc.gpsimd.iota(iotap_i[:], pattern=[[0, 1]], base=0,
                           channel_multiplier=1)
            iotap_f = cpool.tile([P, 1], f32)
            nc.vector.tensor_copy(iotap_f[:], iotap_i[:])

            halfmask = cpool.tile([P, 2], f32)
            nc.vector.tensor_scalar(halfmask[:, 0:1], iotap_f[:], 64.0, None,
                                    op.is_lt)
            nc.vector.tensor_scalar(halfmask[:, 1:2], iotap_f[:], 63.0, None,
                                    op.is_gt)

            wt0, wt1, bias_t = _final_consts(nc, tc, cpool, pt, wt_d, b_d,
                                             mybir)

            for _rep in range(reps):
                for c in range(NCHUNK):
                    r0, r1 = c * P, (c + 1) * P

                    idx_nat = iop.tile([P, L], i32)
                    nc.sync.dma_start(out=idx_nat[:], in_=hid[r0:r1, :])
                    ht = iop.tile([P, L], f32)
                    nc.sync.dma_start(out=ht[:], in_=ht_d[r0:r1, :])
                    ct = iop.tile([P, 1], f32)
                    nc.sync.dma_start(out=ct[:], in_=ct_d[r0:r1, None])
                    hl_i = iop.tile([P, 1], i32)
                    nc.sync.dma_start(out=hl_i[:], in_=hl_d[r0:r1, None])
                    eid = iop.tile([P, 1], i32)
                    nc.sync.dma_start(out=eid[:], in_=ent[r0:r1, None])

                    nct = stp.tile([P, 1], f32)
                    nc.vector.tensor_scalar_mul(nct[:], ct[:], -1.0)
                    hl_f = stp.tile([P, 1], f32)
                    nc.vector.tensor_copy(hl_f[:], hl_i[:])

                    wdup = stp.tile([P, 2 * L], f32)
                    nc.scalar.activation(out=wdup[:, 0:L], in_=ht[:],
                                         func=act.Exp, bias=nct[:], scale=1.0)
                    mask = stp.tile([P, L], f32)
                    nc.vector.tensor_scalar(mask[:], iota64_f[:], hl_f[:],
                                            None, op.is_lt)
                    nc.vector.tensor_tensor(out=wdup[:, 0:L],
                                            in0=wdup[:, 0:L], in1=mask[:],
                                            op=op.mult)
                    m_f = stp.tile([P, 1], f32)
                    nc.vector.tensor_scalar(m_f[:], hl_f[:], 0.0, None,
                                            op.is_equal)
                    nc.vector.tensor_tensor(out=wdup[:, 0:1],
                                            in0=wdup[:, 0:1], in1=m_f[:],
                                            op=op.add)
                    nc.vector.tensor_copy(wdup[:, L:2 * L], wdup[:, 0:L])

                    m_i = stp.tile([P, 1], i32)
                    nc.vector.tensor_scalar(m_i[:], hl_i[:], 0, None,
                                            op.is_equal)
                    nc.vector.copy_predicated(out=idx_nat[:, 0:1],
                                              mask=m_i[:], data=eid[:])

                    idxdup = stp.tile([P, 2 * L], f32)
                    nc.vector.tensor_copy(idxdup[:, 0:L], idx_nat[:])
                    nc.vector.tensor_copy(idxdup[:, L:2 * L], idx_nat[:])

                    t_w = pt.tile([P, P], f32, tag="tw")
                    nc.tensor.transpose(out=t_w[:], in_=wdup[:],
                                        identity=ident[:])
                    t_i = pt.tile([P, P], f32, tag="ti")
                    nc.tensor.transpose(out=t_i[:], in_=idxdup[:],
                                        identity=ident[:])

                    w_shuf = stp.tile([P, L], f32)
                    nc.vector.tensor_copy(w_shuf[0:64, :], t_w[0:64, 0:P:2])
                    nc.vector.tensor_copy(w_shuf[64:P, :], t_w[64:P, 1:P:2])
                    idx_shuf_f = stp.tile([P, L], f32)
                    nc.vector.tensor_copy(idx_shuf_f[0:64, :],
                                          t_i[0:64, 0:P:2])
                    nc.vector.tensor_copy(idx_shuf_f[64:P, :],
                                          t_i[64:P, 1:P:2])
                    idx_shuf = stp.tile([P, L], i32)
                    nc.vector.tensor_copy(idx_shuf[:], idx_shuf_f[:])

                    rhs_full = stp.tile([P, 2 * L], f32)
                    nc.vector.tensor_tensor(
                        out=rhs_full[:].rearrange("p (j n) -> p j n", n=2),
                        in0=w_shuf[:, :, None].to_broadcast([P, L, 2]),
                        in1=halfmask[:, None, :].to_broadcast([P, L, 2]),
                        op=op.mult)

                    hisT0 = phis.tile([P, P], f32)
                    hisT1 = phis.tile([P, P], f32)

                    for J in range(NPAIR):
                        g = gp.tile([P, D], f32, tag="g")
                        nc.gpsimd.indirect_dma_start(
                            out=g[:], out_offset=None, in_=emb[:],
                            in_offset=bass.IndirectOffsetOnAxis(
                                ap=idx_shuf[:, J:J + 1], axis=0))
                        nc.tensor.matmul(
                            out=hisT0[:, 2 * J:2 * J + 2], lhsT=g[:, 0:P],
                            rhs=rhs_full[:, 2 * J:2 * J + 2],
                            start=True, stop=True)
                        nc.tensor.matmul(
                            out=hisT1[:, 2 * J:2 * J + 2], lhsT=g[:, P:D],
                            rhs=rhs_full[:, 2 * J:2 * J + 2],
                            start=True, stop=True)

                    hisT0_sb = outp.tile([P, P], f32)
                    nc.vector.tensor_copy(hisT0_sb[:], hisT0[:])
                    hisT1_sb = outp.tile([P, P], f32)
                    nc.vector.tensor_copy(hisT1_sb[:], hisT1[:])

                    y_ps = py.tile([P, D], f32)
                    nc.tensor.matmul(out=y_ps[:], lhsT=hisT0_sb[:],
                                     rhs=wt0[:], start=True, stop=False)
                    nc.tensor.matmul(out=y_ps[:], lhsT=hisT1_sb[:],
                                     rhs=wt1[:], start=False, stop=True)

                    y_sb = outp.tile([P, D], f32)
                    nc.vector.tensor_tensor(out=y_sb[:], in0=y_ps[:],
                                            in1=bias_t[:], op=op.add)
                    nc.sync.dma_start(out=y_d[r0:r1, :], in_=y_sb[:])

    nc.compile()
    return nc


_NC_CACHE = {}


def _get_nc(which):
    if which not in _NC_CACHE:
        _NC_CACHE[which] = (build_nc_v2() if which == "v2"
                            else build_nc_v1())
    return _NC_CACHE[which]


def _norm_inputs(ent_ids, current_time, hist_ids, hist_times, hist_len,
                 emb, W, b):
    return (
        np.ascontiguousarray(np.asarray(ent_ids, dtype=np.int32)),
        np.ascontiguousarray(np.asarray(current_time, np.float32)),
        np.ascontiguousarray(np.asarray(hist_ids, dtype=np.int32)),
        np.ascontiguousarray(np.asarray(hist_times, np.float32)),
        np.ascontiguousarray(np.asarray(hist_len, dtype=np.int32)),
        np.ascontiguousarray(np.asarray(emb, dtype=np.float32)),
        np.ascontiguousarray(np.asarray(W, dtype=np.float32)),
        np.ascontiguousarray(np.asarray(b, dtype=np.float32)),
    )


def make_in_maps(ent_ids, current_time, hist_ids, hist_times, hist_len,
                 emb, W, b):
    """v1 (dense) per-core input maps."""
    ent_ids, current_time, hist_ids, hist_times, hist_len, emb, W, b = \
        _norm_inputs(ent_ids, current_time, hist_ids, hist_times, hist_len,
                     emb, W, b)
    WT = np.ascontiguousarray(W.T)
    in_maps = []
    for c in range(N_CORES):
        s = slice(c * BL, (c + 1) * BL)
        in_maps.append({
            "ent_ids": ent_ids[s], "current_time": current_time[s],
            "hist_ids": hist_ids[s], "hist_times": hist_times[s],
            "hist_len": hist_len[s], "emb": emb, "WT": WT, "bvec": b,
        })
    return in_maps


def make_in_maps_v2(ent_ids, current_time, hist_ids, hist_times, hist_len,
                    emb, W, b):
    """v2 (ragged) per-core input maps + output permutation, or (None, None)."""
    ent_ids, current_time, hist_ids, hist_times, hist_len, emb, W, b = \
        _norm_inputs(ent_ids, current_time, hist_ids, hist_times, hist_len,
                     emb, W, b)
    packs, order = pack_v2(ent_ids, current_time, hist_ids, hist_times,
                           hist_len)
    if packs is None:
        return None, None
    WT = np.ascontiguousarray(W.T)
    in_maps = []
    for c in range(N_CORES):
        m = dict(packs[c])
        m.update({"emb": emb, "WT": WT, "bvec": b})
        in_maps.append(m)
    return in_maps, order


def kernel(ent_ids, current_time, hist_ids, hist_times, hist_len, emb, W, b):
    from concourse.bass_utils import run_bass_kernel_spmd

    args = (ent_ids, current_time, hist_ids, hist_times, hist_len, emb, W, b)
    in_maps, order = make_in_maps_v2(*args)
    if in_maps is not None:
        nc = _get_nc("v2")
        res = run_bass_kernel_spmd(nc, in_maps, list(range(N_CORES)))
        y_sorted = np.stack([res.results[c]["y"] for c in range(N_CORES)])
        # core c position p holds batch row order[8p + c]
        y_full = np.empty((B, D), np.float32)
        pos = np.arange(BL)
        for c in range(N_CORES):
            y_full[order[N_CORES * pos + c]] = y_sorted[c]
        return y_full

    nc = _get_nc("v1")
    res = run_bass_kernel_spmd(nc, make_in_maps(*args),
                               list(range(N_CORES)))
    return np.concatenate([res.results[c]["y"] for c in range(N_CORES)],
                          axis=0)
